# revision 8
# baseline (speedup 1.0000x reference)
"""BWGNN (Beta-Wavelet GNN) Trainium2 kernel — 8-core SPMD.

Math (exact refactoring of the reference):
  h  = relu(relu(x@W1+b1)@W2+b2)
  P(f) = f - dinv * segsum_dst((f*dinv)[src])          (dinv = clip(deg,1)^-0.5)
  All 3 Beta-Bernstein filters are polynomials of the same operator P applied
  to the same h, so only p1=P(h), p2=P(p1) are needed (2 message rounds):
    concat_i(sum_k theta_ik P^k h) @ Wm1 = h@A0 + p1@A1 + p2@A2,
    A_k = sum_i theta_ik * Wm1[64i:64(i+1)]            (host-precomputed)
  out = relu(h@A0 + p1@A1 + p2@A2 + bm1) @ Wm2 + bm2

Distribution: nodes block-sharded over 8 cores (12500/core); edges partitioned
by destination core; per-round AllGather of the scaled features (f*dinv) so
each core gathers source rows locally from its replicated table.

Message aggregation (the hot loop): per-core nodes are sorted into "positions"
by (in-degree from cores 0-3, in-degree from cores 4-7) lex key; stripe
gs = 128 consecutive positions. The grid cell (gs, k, p) holds the k-th
in-edge of the node at position gs*128+p, split into bank A (src core 0-3)
and bank B (src core 4-7) column groups. Each (bank, stripe-batch) is ONE
InstDMAGatherAnt: int16 indices are sign-extended by the Q7 ucode, so with
the table base planted mid-window each gather addresses +-32768 rows — bank A
covers gid [0,50176) from base 32768, bank B covers [50176,100352) from base
67584. Pad cells point at zeroed table rows (positive idx so the ucode's
trailing-negative trim never eats real cells; one all-pad column is appended
per instruction to guarantee a non-negative final entry). Per-stripe
tensor_reduce over the gathered [128, K*64] block then finishes the segment
sum on DVE. Desc-gen for banks A/B runs on separate SWDGE queues (Q7 pairs).
"""
import hashlib
import os

import numpy as np

import concourse.bass as bass
import concourse.mybir as mb
import concourse.tile as tile
from concourse import bacc, bass2jax
from concourse.masks import make_identity

# ---------------- problem constants (hardcoded per contract) ----------------
N_NODES = 100000
N_EDGES = 1600000
IN_FEATS = 128
H = 64
NUM_CLASSES = 2
N_CORES = 8
NPC = N_NODES // N_CORES            # 12500 nodes per core
P = 128
S_ALL = (NPC + P - 1) // P          # 98 stripes
NPC_PAD = S_ALL * P                 # 12544
NTAB = N_CORES * NPC_PAD            # 100352 rows in the gathered table
F32 = mb.dt.float32
I16 = mb.dt.int16

BASE_A = 32768                      # bank A table base row (covers [0, 65536))
BASE_B = 67584                      # bank B base row (covers [34816, 100352))
PAD_A = 3 * NPC_PAD + NPC           # 50132: zeroed row, idx 17364 > 0
PAD_B = 6 * NPC_PAD + NPC           # 87764: zeroed row, idx 20180 > 0
COL_BUDGET = 64                     # grid columns per gather instruction

THETAS = np.array([[3.0, -3.0, 0.75],
                   [0.0, 3.0, -1.5],
                   [0.0, 0.0, 0.75]])  # [filter, power]  (Beta-Bernstein, D=2)

_NEFF_CACHE_DIR = os.environ.get("BASS_NEFF_CACHE", "/tmp/neff_cache")


def _install_neff_cache():
    """Disk-cache walrus compiles by BIR hash (no cache in the stock hook)."""
    import concourse.bass_utils as bass_utils
    if getattr(bass2jax, "_neff_cache_installed", False):
        return
    orig = bass_utils.compile_bir_kernel

    def cached(bir_json, tmpdir, neff_name="file.neff"):
        os.makedirs(_NEFF_CACHE_DIR, exist_ok=True)
        key = hashlib.sha256(bir_json).hexdigest()[:32]
        path = os.path.join(_NEFF_CACHE_DIR, f"{key}.neff")
        if os.path.exists(path):
            dst = os.path.join(tmpdir, neff_name)
            with open(path, "rb") as f, open(dst, "wb") as g:
                g.write(f.read())
            return dst
        out = orig(bir_json, tmpdir, neff_name)
        try:
            with open(out, "rb") as f, open(path + ".tmp", "wb") as g:
                g.write(f.read())
            os.replace(path + ".tmp", path)
        except OSError:
            pass
        return out

    bass_utils.compile_bir_kernel = cached
    bass2jax.compile_bir_kernel = cached
    bass2jax._neff_cache_installed = True


# ---------------- walrus 1-wait-per-instruction workaround ----------------
def _split_waits(nc):
    """This walrus build rejects >1 sync wait per instruction; move excess
    waits onto no-fuse nops inserted just before, on the same engine."""
    for bb in nc.main_func.blocks:
        insts = list(bb.instructions)
        out, changed = [], False
        for inst in insts:
            si = inst.sync_info
            waits = list(si.on_wait) if si and si.on_wait else []
            if len(waits) > 1:
                for i, w in enumerate(waits[:-1]):
                    out.append(mb.InstNoOp(
                        name=f"{inst.name}-ws{i}", bass_nofuse=True,
                        engine=inst.engine,
                        sync_info=mb.SyncInfo(on_wait=[w], on_update=[])))
                si.on_wait = waits[-1:]
                inst.sync_info = si
                changed = True
            out.append(inst)
        if changed:
            bb.instructions = out


def _wrap16(v):
    """Grid idx [128, S] (partition, col) -> ucode idx layout [128, S*8]:
    list position j = col*128 + p lives at [j%16, j//16], replicated x8."""
    Ptot, S = v.shape
    w = v.reshape(8, 16, S).transpose(1, 2, 0).reshape(16, S * 8)
    return np.ascontiguousarray(np.tile(w, (8, 1)))


# ---------------- host-side preprocessing ----------------
def preprocess(x, edge_index):
    src = np.asarray(edge_index[0], dtype=np.int64)
    dst = np.asarray(edge_index[1], dtype=np.int64)
    x = np.asarray(x, dtype=np.float32)

    deg = np.bincount(dst, minlength=N_NODES).astype(np.int64)
    bankB = src >= (N_NODES // 2)                      # src core 4-7
    degA = np.bincount(dst[~bankB], minlength=N_NODES).astype(np.int64)
    degB = deg - degA

    # per-core position assignment: lex sort by (degA, degB) descending
    pos = np.empty(N_NODES, dtype=np.int64)
    orders = []
    for c in range(N_CORES):
        sl = slice(c * NPC, (c + 1) * NPC)
        key = degA[sl] * 1024 + degB[sl]
        order = np.argsort(-key, kind="stable")        # position -> local node
        orders.append(order)
        pos[c * NPC + order] = np.arange(NPC)
    gid = (np.arange(N_NODES) // NPC) * NPC_PAD + pos  # node -> table row

    # shared per-stripe slot counts (max over cores, so SPMD grid is uniform)
    def stripe_max(dvec):
        K = np.zeros(S_ALL, dtype=np.int64)
        for c in range(N_CORES):
            d = dvec[c * NPC:(c + 1) * NPC][orders[c]]
            d = np.r_[d, np.zeros(NPC_PAD - NPC, np.int64)]
            K = np.maximum(K, d.reshape(S_ALL, P).max(axis=1))
        return K

    KA = stripe_max(degA)
    KB = stripe_max(degB)

    # batches of stripes per gather instruction (per-bank column budget)
    batches = []
    cur, sa, sb = [], 0, 0
    for gs in range(S_ALL):
        if cur and (sa + KA[gs] > COL_BUDGET or sb + KB[gs] > COL_BUDGET):
            batches.append((tuple(cur), sa, sb))
            cur, sa, sb = [], 0, 0
        cur.append(gs)
        sa += int(KA[gs])
        sb += int(KB[gs])
    batches.append((tuple(cur), sa, sb))

    # global column base per stripe, +1 pad column at the end of each batch
    colA = np.zeros(S_ALL, dtype=np.int64)
    colB = np.zeros(S_ALL, dtype=np.int64)
    ca = cb = 0
    for stripes, _, _ in batches:
        for gs in stripes:
            colA[gs] = ca
            colB[gs] = cb
            ca += int(KA[gs])
            cb += int(KB[gs])
        ca += 1  # pad column
        cb += 1
    SA_TOT, SB_TOT = int(ca), int(cb)

    # per-edge rank within (dst, bank)
    rank = np.empty(N_EDGES, dtype=np.int64)
    for mask in (~bankB, bankB):
        ds = dst[mask]
        eo = np.argsort(ds, kind="stable")
        d_sorted = ds[eo]
        first = np.r_[0, np.flatnonzero(np.diff(d_sorted)) + 1]
        run_id = np.zeros(len(ds), dtype=np.int64)
        run_id[first[1:]] = 1
        run_id = np.cumsum(run_id)
        r = np.arange(len(ds)) - first[run_id]
        rr = np.empty(len(ds), dtype=np.int64)
        rr[eo] = r
        rank[mask] = rr

    q = pos[dst]
    gs_e = q // P
    row_e = q % P
    core_e = dst // NPC
    colA_e = colA[gs_e] + rank
    colB_e = colB[gs_e] + rank

    idxA = np.full((N_CORES, P, SA_TOT), PAD_A - BASE_A, dtype=np.int16)
    idxB = np.full((N_CORES, P, SB_TOT), PAD_B - BASE_B, dtype=np.int16)
    mA = ~bankB
    idxA[core_e[mA], row_e[mA], colA_e[mA]] = (gid[src[mA]] - BASE_A).astype(np.int16)
    idxB[core_e[bankB], row_e[bankB], colB_e[bankB]] = (gid[src[bankB]] - BASE_B).astype(np.int16)

    idxA_w = np.stack([_wrap16(idxA[c]) for c in range(N_CORES)])
    idxB_w = np.stack([_wrap16(idxB[c]) for c in range(N_CORES)])

    # per-core xT (position order, padded) and deg tile [128, S_ALL]
    deg_pc = deg.reshape(N_CORES, NPC)
    xT = np.zeros((N_CORES, P, NPC_PAD), dtype=np.float32)
    degt = np.ones((N_CORES, P, S_ALL), dtype=np.float32)
    for c in range(N_CORES):
        xc = x[c * NPC:(c + 1) * NPC][orders[c]]          # [NPC, IN]
        xT[c, :, :NPC] = xc.T
        dp = np.ones(NPC_PAD, dtype=np.float32)
        dp[:NPC] = deg_pc[c][orders[c]]
        degt[c] = dp.reshape(S_ALL, P).T                  # deg at (p, s), q=s*128+p

    return dict(idxA=idxA_w, idxB=idxB_w, xT=xT, degt=degt,
                KA=tuple(int(k) for k in KA), KB=tuple(int(k) for k in KB),
                batches=tuple(batches), SA=SA_TOT, SB=SB_TOT,
                colA=colA, colB=colB, orders=orders)


def host_weights(W1, b1, W2, b2, Wm1, bm1, Wm2, bm2):
    A = [sum(float(THETAS[i, k]) * np.asarray(Wm1, np.float32)[i * H:(i + 1) * H, :]
             for i in range(3)) for k in range(3)]
    return dict(
        W1=np.asarray(W1, np.float32), W2=np.asarray(W2, np.float32),
        A0=A[0].astype(np.float32), A1=A[1].astype(np.float32), A2=A[2].astype(np.float32),
        Wm2=np.asarray(Wm2, np.float32),
        b1=np.asarray(b1, np.float32).reshape(H, 1),
        b2=np.asarray(b2, np.float32).reshape(H, 1),
        bm1=np.asarray(bm1, np.float32).reshape(H, 1),
        bm2=np.asarray(bm2, np.float32).reshape(NUM_CLASSES, 1),
    )


# ---------------- device program ----------------
def build_nc(KA, KB, batches, SA_TOT, SB_TOT, reps=1):
    nb_lim = int(os.environ.get("BWGNN_NB", "9999"))
    nc = bacc.Bacc(None, target_bir_lowering=False, num_swdge_queues=1)
    dp = nc.declare_dram_parameter
    xT_d = dp("xT", [P, NPC_PAD], F32, isOutput=False)
    deg_d = dp("degt", [P, S_ALL], F32, isOutput=False)
    idxA_d = dp("idxA", [P, SA_TOT * 8], I16, isOutput=False)
    idxB_d = dp("idxB", [P, SB_TOT * 8], I16, isOutput=False)
    W1_d = dp("W1", [IN_FEATS, H], F32, isOutput=False)
    W2_d = dp("W2", [H, H], F32, isOutput=False)
    A0_d = dp("A0", [H, H], F32, isOutput=False)
    A1_d = dp("A1", [H, H], F32, isOutput=False)
    A2_d = dp("A2", [H, H], F32, isOutput=False)
    Wm2_d = dp("Wm2", [H, NUM_CLASSES], F32, isOutput=False)
    b1_d = dp("b1", [H, 1], F32, isOutput=False)
    b2_d = dp("b2", [H, 1], F32, isOutput=False)
    bm1_d = dp("bm1", [H, 1], F32, isOutput=False)
    bm2_d = dp("bm2", [NUM_CLASSES, 1], F32, isOutput=False)
    out_d = dp("outT", [NUM_CLASSES, NPC_PAD], F32, isOutput=True)

    fs_in = [nc.dram_tensor(f"fs{r}_in", [NPC_PAD, H], F32) for r in range(2)]
    fs_full = [nc.dram_tensor(f"fs{r}_full", [NTAB, H], F32, addr_space="Shared")
               for r in range(2)]
    groups = [list(range(N_CORES))]

    with tile.TileContext(nc) as tc:
        with (
            tc.tile_pool(name="const", bufs=1) as cp,
            tc.tile_pool(name="big", bufs=1) as bp,
            tc.tile_pool(name="work", bufs=2) as wp,
            tc.tile_pool(name="ga", bufs=2) as gpa,
            tc.tile_pool(name="gb", bufs=2) as gpb,
            tc.tile_pool(name="ps", bufs=4, space="PSUM") as ps,
        ):
            # ---- constant loads ----
            W1_t = cp.tile([IN_FEATS, H], F32)
            nc.sync.dma_start(out=W1_t[:], in_=W1_d[:])
            W2_t = cp.tile([H, H], F32)
            nc.sync.dma_start(out=W2_t[:], in_=W2_d[:])
            A_t = []
            for i, d in enumerate((A0_d, A1_d, A2_d)):
                a = cp.tile([H, H], F32, name=f"A{i}_t")
                nc.sync.dma_start(out=a[:], in_=d[:])
                A_t.append(a)
            Wm2_t = cp.tile([H, NUM_CLASSES], F32)
            nc.sync.dma_start(out=Wm2_t[:], in_=Wm2_d[:])
            bias = {}
            for nm, d, pp in (("b1", b1_d, H), ("b2", b2_d, H),
                              ("bm1", bm1_d, H), ("bm2", bm2_d, NUM_CLASSES)):
                t = cp.tile([pp, 1], F32, name=f"{nm}_t")
                nc.sync.dma_start(out=t[:], in_=d[:])
                bias[nm] = t
            idxA_t = cp.tile([P, SA_TOT * 8], I16)
            nc.sync.dma_start(out=idxA_t[:], in_=idxA_d[:])
            idxB_t = cp.tile([P, SB_TOT * 8], I16)
            nc.sync.dma_start(out=idxB_t[:], in_=idxB_d[:])
            ident = cp.tile([P, P], F32)
            make_identity(nc, ident[:])
            zero_t = cp.tile([P, H], F32)
            nc.vector.memset(zero_t[:], 0.0)

            # dinv = 1/sqrt(max(deg,1))
            deg_t = cp.tile([P, S_ALL], F32)
            nc.sync.dma_start(out=deg_t[:], in_=deg_d[:])
            dinv = cp.tile([P, S_ALL], F32)
            nc.vector.tensor_scalar_max(deg_t[:], deg_t[:], 1.0)
            nc.scalar.sqrt(dinv[:], deg_t[:])
            nc.vector.reciprocal(dinv[:], dinv[:])

            h2 = bp.tile([P, S_ALL * H], F32)
            p1 = bp.tile([P, S_ALL * H], F32)
            p2 = bp.tile([P, S_ALL * H], F32)

            for _rep in range(reps):

                # ---- phase 1: h2 = relu(relu(x@W1+b1)@W2+b2), fs0 = h2*dinv ----
                c0 = 0
                while c0 < NPC_PAD:
                    cw = min(512, NPC_PAD - c0)
                    xc = wp.tile([P, cw], F32, name="xc")
                    nc.sync.dma_start(out=xc[:], in_=xT_d[:, c0:c0 + cw])
                    ps1 = ps.tile([H, cw], F32, name="ps1", tag="mm")
                    nc.tensor.matmul(ps1[:], W1_t[:], xc[:], start=True, stop=True)
                    h1c = wp.tile([H, cw], F32, name="h1c")
                    nc.scalar.activation(h1c[:], ps1[:],
                                         mb.ActivationFunctionType.Relu,
                                         bias=bias["b1"][:, 0:1])
                    ps2 = ps.tile([H, cw], F32, name="ps2", tag="mm")
                    nc.tensor.matmul(ps2[:], W2_t[:], h1c[:], start=True, stop=True)
                    h2c = wp.tile([H, cw], F32, name="h2c")
                    nc.scalar.activation(h2c[:], ps2[:],
                                         mb.ActivationFunctionType.Relu,
                                         bias=bias["b2"][:, 0:1])
                    for s in range(cw // P):
                        gs = (c0 // P) + s
                        pst = ps.tile([P, H], F32, name="pst", tag="tr")
                        nc.tensor.transpose(pst[:], h2c[:, s * P:(s + 1) * P],
                                            ident[:H, :H])
                        nc.vector.tensor_copy(h2[:, gs * H:(gs + 1) * H], pst[:])
                        fst = wp.tile([P, H], F32, name="fst")
                        nc.vector.tensor_scalar_mul(fst[:], pst[:],
                                                    dinv[:, gs:gs + 1])
                        nc.sync.dma_start(out=fs_in[0][gs * P:(gs + 1) * P, :],
                                          in_=fst[:])
                        if gs == S_ALL - 1 and NPC_PAD > NPC:
                            nc.sync.dma_start(
                                out=fs_in[0][NPC:NPC_PAD, :],
                                in_=zero_t[:NPC_PAD - NPC, :])
                    c0 += cw

                nc.gpsimd.collective_compute(
                    "AllGather", mb.AluOpType.bypass, replica_groups=groups,
                    ins=[fs_in[0][:]], outs=[fs_full[0][:]])

                # ---- rounds ----
                for rnd in range(2):
                    tab = fs_full[rnd]
                    p_prev = h2 if rnd == 0 else p1
                    p_out = p1 if rnd == 0 else p2
                    ca = cb = 0
                    for bi, (stripes, sa, sb) in enumerate(batches):
                        if bi >= nb_lim:
                            for gs in stripes:
                                sl = slice(gs * H, (gs + 1) * H)
                                nc.vector.tensor_copy(p_out[:, sl], p_prev[:, sl])
                                if rnd == 0:
                                    fst = wp.tile([P, H], F32, name="fs1t")
                                    nc.vector.tensor_scalar_mul(
                                        fst[:], p_out[:, sl], dinv[:, gs:gs + 1])
                                    nc.sync.dma_start(
                                        out=fs_in[1][gs * P:(gs + 1) * P, :],
                                        in_=fst[:])
                                    if gs == S_ALL - 1 and NPC_PAD > NPC:
                                        nc.sync.dma_start(
                                            out=fs_in[1][NPC:NPC_PAD, :],
                                            in_=zero_t[:NPC_PAD - NPC, :])
                            ca += sa + 1
                            cb += sb + 1
                            continue
                        gbA = gpa.tile([P, (COL_BUDGET + 1) * H], F32, name="gbA")
                        gbB = gpb.tile([P, (COL_BUDGET + 1) * H], F32, name="gbB")
                        nA, nB = (sa + 1) * P, (sb + 1) * P
                        nc.gpsimd.dma_gather(
                            out_ap=gbA[:, :(sa + 1) * H].rearrange(
                                "p (c f) -> p c f", f=H),
                            in_ap=tab[BASE_A:BASE_A + 2, :],
                            idxs_ap=idxA_t[:, ca * 8:(ca + sa + 1) * 8],
                            num_idxs=nA, num_idxs_reg=nA, elem_size=H,
                            queue_num=0)
                        nc.gpsimd.dma_gather(
                            out_ap=gbB[:, :(sb + 1) * H].rearrange(
                                "p (c f) -> p c f", f=H),
                            in_ap=tab[BASE_B:BASE_B + 2, :],
                            idxs_ap=idxB_t[:, cb * 8:(cb + sb + 1) * 8],
                            num_idxs=nB, num_idxs_reg=nB, elem_size=H,
                            queue_num=0)
                        oa, ob = 0, 0
                        for gs in stripes:
                            ka, kb = KA[gs], KB[gs]
                            sl = slice(gs * H, (gs + 1) * H)
                            dcol = dinv[:, gs:gs + 1]
                            red = wp.tile([P, H], F32, name="red")
                            redB = wp.tile([P, H], F32, name="redB")
                            if ka > 0:
                                nc.vector.tensor_reduce(
                                    out=red[:],
                                    in_=gbA[:, oa * H:(oa + ka) * H].rearrange(
                                        "p (k f) -> p f k", f=H),
                                    axis=mb.AxisListType.X, op=mb.AluOpType.add)
                            else:
                                nc.vector.memset(red[:], 0.0)
                            if kb > 0:
                                nc.vector.tensor_reduce(
                                    out=redB[:],
                                    in_=gbB[:, ob * H:(ob + kb) * H].rearrange(
                                        "p (k f) -> p f k", f=H),
                                    axis=mb.AxisListType.X, op=mb.AluOpType.add)
                                nc.vector.tensor_tensor(red[:], red[:], redB[:],
                                                        op=mb.AluOpType.add)
                            nc.vector.tensor_scalar_mul(red[:], red[:], dcol)
                            nc.vector.tensor_tensor(p_out[:, sl], p_prev[:, sl],
                                                    red[:],
                                                    op=mb.AluOpType.subtract)
                            if rnd == 0:
                                fst = wp.tile([P, H], F32, name="fs1t")
                                nc.vector.tensor_scalar_mul(fst[:], p_out[:, sl],
                                                            dcol)
                                nc.sync.dma_start(
                                    out=fs_in[1][gs * P:(gs + 1) * P, :],
                                    in_=fst[:])
                                if gs == S_ALL - 1 and NPC_PAD > NPC:
                                    nc.sync.dma_start(
                                        out=fs_in[1][NPC:NPC_PAD, :],
                                        in_=zero_t[:NPC_PAD - NPC, :])
                            oa += ka
                            ob += kb
                        ca += sa + 1
                        cb += sb + 1
                    if rnd == 0:
                        nc.gpsimd.collective_compute(
                            "AllGather", mb.AluOpType.bypass, replica_groups=groups,
                            ins=[fs_in[1][:]], outs=[fs_full[1][:]])

                # ---- final: out = relu(h2@A0+p1@A1+p2@A2+bm1)@Wm2+bm2 ----
                for gs in range(S_ALL):
                    sl = slice(gs * H, (gs + 1) * H)
                    psy = ps.tile([H, P], F32, name="psy", tag="mm")
                    for i, srcbuf in enumerate((h2, p1, p2)):
                        pst = ps.tile([H, P], F32, name="ftr", tag="tr")
                        nc.tensor.transpose(pst[:], srcbuf[:, sl], ident[:])
                        rhs = wp.tile([H, P], F32, name="frhs")
                        nc.vector.tensor_copy(rhs[:], pst[:])
                        nc.tensor.matmul(psy[:], A_t[i][:], rhs[:],
                                         start=(i == 0), stop=(i == 2))
                    y2 = wp.tile([H, P], F32, name="y2")
                    nc.scalar.activation(y2[:], psy[:],
                                         mb.ActivationFunctionType.Relu,
                                         bias=bias["bm1"][:, 0:1])
                    pso = ps.tile([NUM_CLASSES, P], F32, name="pso", tag="mm")
                    nc.tensor.matmul(pso[:], Wm2_t[:], y2[:], start=True, stop=True)
                    ot = wp.tile([NUM_CLASSES, P], F32, name="ot")
                    nc.vector.tensor_scalar_add(ot[:], pso[:], bias["bm2"][:, 0:1])
                    nc.sync.dma_start(out=out_d[:, gs * P:(gs + 1) * P], in_=ot[:])

    nc.finalize()
    return nc


# ---------------- execution (axon PJRT, 8 devices) ----------------
class _Exec:
    def __init__(self, nc):
        import jax
        from jax.sharding import Mesh, PartitionSpec, NamedSharding
        from jax.experimental.shard_map import shard_map
        _install_neff_cache()
        bass2jax.install_neuronx_cc_hook()
        self.jax = jax
        pn = nc.partition_id_tensor.name if nc.partition_id_tensor else None
        in_names, out_names, out_avals = [], [], []
        for alloc in nc.m.functions[0].allocations:
            if not isinstance(alloc, mb.MemoryLocationSet):
                continue
            name = alloc.memorylocations[0].name
            if alloc.kind == "ExternalInput":
                if name != pn:
                    in_names.append(name)
            elif alloc.kind == "ExternalOutput":
                out_names.append(name)
                out_avals.append(jax.core.ShapedArray(
                    tuple(alloc.tensor_shape), mb.dt.np(alloc.dtype)))
        self.in_names, self.out_names, self.out_avals = in_names, out_names, out_avals
        n_params, n_outs = len(in_names), len(out_avals)
        all_in = list(in_names) + list(out_names)
        if pn is not None:
            all_in.append(pn)

        def _body(*args):
            operands = list(args)
            if pn is not None:
                operands.append(bass2jax.partition_id_tensor())
            return tuple(bass2jax._bass_exec_p.bind(
                *operands, out_avals=tuple(out_avals), in_names=tuple(all_in),
                out_names=tuple(out_names), lowering_input_output_aliases=(),
                sim_require_finite=False, sim_require_nnan=False, nc=nc))

        devices = jax.devices()[:N_CORES]
        mesh = Mesh(np.asarray(devices), ("core",))
        self.fn = jax.jit(
            shard_map(_body, mesh=mesh,
                      in_specs=(PartitionSpec("core"),) * (n_params + n_outs),
                      out_specs=(PartitionSpec("core"),) * n_outs,
                      check_rep=False),
            donate_argnums=tuple(range(n_params, n_params + n_outs)),
            keep_unused=True)
        self.sharding = NamedSharding(mesh, PartitionSpec("core"))

    def put(self, in_maps):
        arrs = [np.concatenate([np.asarray(m[n]) for m in in_maps], axis=0)
                for n in self.in_names]
        return [self.jax.device_put(a, self.sharding) for a in arrs]

    def run(self, dev_in):
        zo = [self.jax.device_put(
            np.zeros((N_CORES * a.shape[0], *a.shape[1:]), a.dtype), self.sharding)
            for a in self.out_avals]
        outs = self.fn(*dev_in, *zo)
        self.jax.block_until_ready(outs)
        return outs

    def fetch(self, outs):
        return [np.asarray(o).reshape(N_CORES, *self.out_avals[i].shape)
                for i, o in enumerate(outs)]


_CACHE = {}


def _prepare(x, edge_index, W1, b1, W2, b2, Wm1, bm1, Wm2, bm2, reps=1):
    pre = preprocess(x, edge_index)
    wts = host_weights(W1, b1, W2, b2, Wm1, bm1, Wm2, bm2)
    key = ("nc", pre["KA"], pre["KB"], pre["batches"], reps,
           os.environ.get("BWGNN_NB", "9999"))
    if key not in _CACHE:
        nc = build_nc(pre["KA"], pre["KB"], pre["batches"],
                      pre["SA"], pre["SB"], reps=reps)
        _split_waits(nc)
        _CACHE[key] = _Exec(nc)
    ex = _CACHE[key]
    in_maps = []
    for c in range(N_CORES):
        m = dict(xT=pre["xT"][c], degt=pre["degt"][c],
                 idxA=pre["idxA"][c], idxB=pre["idxB"][c], **wts)
        in_maps.append(m)
    return ex, in_maps, pre


def kernel(x, edge_index, W1, b1, W2, b2, Wm1, bm1, Wm2, bm2):
    ex, in_maps, pre = _prepare(x, edge_index, W1, b1, W2, b2,
                                Wm1, bm1, Wm2, bm2)
    dev_in = ex.put(in_maps)
    outs = ex.run(dev_in)
    outT = ex.fetch(outs)[0]          # [N_CORES, 2, NPC_PAD]
    y = np.empty((N_NODES, NUM_CLASSES), dtype=np.float32)
    for c in range(N_CORES):
        y[c * NPC + pre["orders"][c]] = outT[c, :, :NPC].T
    return y


# revision 10
# speedup vs baseline: 1.0748x; 1.0748x over previous
"""BWGNN (Beta-Wavelet GNN) Trainium2 kernel — 8-core SPMD.

Math (exact refactoring of the reference):
  h  = relu(relu(x@W1+b1)@W2+b2)
  P(f) = f - dinv * segsum_dst((f*dinv)[src])          (dinv = clip(deg,1)^-0.5)
  All 3 Beta-Bernstein filters are polynomials of the same operator P applied
  to the same h, so only p1=P(h), p2=P(p1) are needed (2 message rounds, not 6):
    concat_i(sum_k theta_ik P^k h) @ Wm1 = h@A0 + p1@A1 + p2@A2,
    A_k = sum_i theta_ik * Wm1[64i:64(i+1)]            (host-precomputed)
  out = relu(h@A0 + p1@A1 + p2@A2 + bm1) @ Wm2 + bm2

Distribution: nodes block-sharded over 8 cores (12500/core); edges partitioned
by destination core; per-round AllGather of the scaled features (f*dinv) so
each core gathers source rows locally from its replicated table.

Message aggregation: per-core nodes are degree-sorted into "positions";
slot k holds the k-th edge of every node with deg>k (a position-prefix).
Each (slot, stripe-of-128-positions) is one canonical indirect DMA
(idx [128,1] -> rows [128,64]) with CCE fp32 accumulate into SBUF; padding
entries index a zeroed pad row of the table (adds exact zeros). Two buffer chains (even/odd slots) keep the
qPoolDynamic queue busy; a final add combines them.
"""
import hashlib
import math
import os
import time

import numpy as np

import concourse.bass as bass
import concourse.mybir as mb
import concourse.tile as tile
from concourse import bass2jax
from concourse.masks import make_identity

# ---------------- problem constants (hardcoded per contract) ----------------
N_NODES = 100000
N_EDGES = 1600000
IN_FEATS = 128
H = 64
NUM_CLASSES = 2
N_CORES = 8
NPC = N_NODES // N_CORES            # 12500 nodes per core
P = 128
S_ALL = (NPC + P - 1) // P          # 98 stripes
NPC_PAD = S_ALL * P                 # 12544
NTAB = N_CORES * NPC_PAD            # 100352 rows in the gathered table
SENTINEL = NPC                      # pad-row table index (those rows are zeroed)
F32 = mb.dt.float32
I32 = mb.dt.int32

THETAS = np.array([[3.0, -3.0, 0.75],
                   [0.0, 3.0, -1.5],
                   [0.0, 0.0, 0.75]])  # [filter, power]  (Beta-Bernstein, D=2)

_NEFF_CACHE_DIR = os.environ.get("BASS_NEFF_CACHE", "/tmp/neff_cache")


def _install_neff_cache():
    """Disk-cache walrus compiles by BIR hash (no cache in the stock hook)."""
    import concourse.bass_utils as bass_utils
    if getattr(bass2jax, "_neff_cache_installed", False):
        return
    orig = bass_utils.compile_bir_kernel

    def cached(bir_json, tmpdir, neff_name="file.neff"):
        os.makedirs(_NEFF_CACHE_DIR, exist_ok=True)
        key = hashlib.sha256(bir_json).hexdigest()[:32]
        path = os.path.join(_NEFF_CACHE_DIR, f"{key}.neff")
        if os.path.exists(path):
            dst = os.path.join(tmpdir, neff_name)
            with open(path, "rb") as f, open(dst, "wb") as g:
                g.write(f.read())
            return dst
        out = orig(bir_json, tmpdir, neff_name)
        try:
            with open(out, "rb") as f, open(path + ".tmp", "wb") as g:
                g.write(f.read())
            os.replace(path + ".tmp", path)
        except OSError:
            pass
        return out

    bass_utils.compile_bir_kernel = cached
    bass2jax.compile_bir_kernel = cached
    bass2jax._neff_cache_installed = True


# ---------------- walrus 1-wait-per-instruction workaround ----------------
def _split_waits(nc):
    """This walrus build rejects >1 sync wait per instruction; move excess
    waits onto no-fuse nops inserted just before, on the same engine."""
    for bb in nc.main_func.blocks:
        insts = list(bb.instructions)
        out, changed = [], False
        for inst in insts:
            si = inst.sync_info
            waits = list(si.on_wait) if si and si.on_wait else []
            if len(waits) > 1:
                for i, w in enumerate(waits[:-1]):
                    out.append(mb.InstNoOp(
                        name=f"{inst.name}-ws{i}", bass_nofuse=True,
                        engine=inst.engine,
                        sync_info=mb.SyncInfo(on_wait=[w], on_update=[])))
                si.on_wait = waits[-1:]
                inst.sync_info = si
                changed = True
            out.append(inst)
        if changed:
            bb.instructions = out


# ---------------- host-side preprocessing ----------------
def preprocess(x, edge_index):
    """Degree-sort nodes per core, build slot/stripe gather indices.

    Returns dict with per-core arrays + shared structure.
    """
    src = np.asarray(edge_index[0], dtype=np.int64)
    dst = np.asarray(edge_index[1], dtype=np.int64)
    x = np.asarray(x, dtype=np.float32)

    deg = np.bincount(dst, minlength=N_NODES).astype(np.int64)

    # per-core degree sort -> positions
    pos = np.empty(N_NODES, dtype=np.int64)
    orders = []
    for c in range(N_CORES):
        dc = deg[c * NPC:(c + 1) * NPC]
        order = np.argsort(-dc, kind="stable")      # position -> local node
        orders.append(order)
        pos[c * NPC + order] = np.arange(NPC)
    gid = (np.arange(N_NODES) // NPC) * NPC_PAD + pos   # node -> table row

    # edge rank within destination
    eorder = np.argsort(dst, kind="stable")
    ds = dst[eorder]
    first = np.r_[0, np.flatnonzero(np.diff(ds)) + 1]
    run_id = np.zeros(N_EDGES, dtype=np.int64)
    run_id[first[1:]] = 1
    run_id = np.cumsum(run_id)
    rank_sorted = np.arange(N_EDGES) - first[run_id]
    rank = np.empty(N_EDGES, dtype=np.int64)
    rank[eorder] = rank_sorted

    # shared slot structure (max over cores)
    deg_pc = deg.reshape(N_CORES, NPC)
    kmax = int(deg.max())
    ks = np.arange(kmax)
    n_k = (deg_pc[:, None, :] > ks[None, :, None]).sum(axis=2).max(axis=0)  # [kmax]
    s_k = np.maximum(1, (n_k + P - 1) // P).astype(np.int64)                # stripes per slot
    cum = np.r_[0, np.cumsum(s_k)]
    tot_s = int(cum[-1])

    # per-edge placement
    q = pos[dst]
    col = cum[rank] + q // P
    row = q % P
    core_e = dst // NPC
    idx_all = np.full((N_CORES, P, tot_s), SENTINEL, dtype=np.int32)
    idx_all[core_e, row, col] = gid[src].astype(np.int32)

    # per-core xT (position order, padded) and deg tile [128, S_ALL]
    xT = np.zeros((N_CORES, P, NPC_PAD), dtype=np.float32)
    degt = np.ones((N_CORES, P, S_ALL), dtype=np.float32)
    for c in range(N_CORES):
        xc = x[c * NPC:(c + 1) * NPC][orders[c]]          # [NPC, IN]
        xT[c, :, :NPC] = xc.T
        dp = np.ones(NPC_PAD, dtype=np.float32)
        dp[:NPC] = deg_pc[c][orders[c]]
        degt[c] = dp.reshape(S_ALL, P).T                  # deg at (p, s) = q=s*128+p

    return dict(idx=idx_all, xT=xT, degt=degt, s_k=s_k, cum=cum,
                tot_s=tot_s, kmax=kmax, orders=orders)


def host_weights(W1, b1, W2, b2, Wm1, bm1, Wm2, bm2):
    A = [sum(float(THETAS[i, k]) * np.asarray(Wm1, np.float32)[i * H:(i + 1) * H, :]
             for i in range(3)) for k in range(3)]
    return dict(
        W1=np.asarray(W1, np.float32), W2=np.asarray(W2, np.float32),
        A0=A[0].astype(np.float32), A1=A[1].astype(np.float32), A2=A[2].astype(np.float32),
        Wm2=np.asarray(Wm2, np.float32),
        b1=np.asarray(b1, np.float32).reshape(H, 1),
        b2=np.asarray(b2, np.float32).reshape(H, 1),
        bm1=np.asarray(bm1, np.float32).reshape(H, 1),
        bm2=np.asarray(bm2, np.float32).reshape(NUM_CLASSES, 1),
    )


# ---------------- device program ----------------
def build_nc(s_k, cum, tot_s, reps=1, bf16=False):
    """Build the SPMD Bass program. Slot structure (s_k stripes per slot) is
    compile-time static and identical on all cores.

    bf16=True stores the gathered feature tables (fs*) in bfloat16 — halves
    gather + allgather traffic; accumulation stays fp32 via CCE cast-add."""
    nc = bass.Bass()
    TDT = mb.dt.bfloat16 if bf16 else F32
    dp = nc.declare_dram_parameter
    xT_d = dp("xT", [P, NPC_PAD], F32, isOutput=False)
    deg_d = dp("degt", [P, S_ALL], F32, isOutput=False)
    idx_d = dp("idx", [P, tot_s], I32, isOutput=False)
    W1_d = dp("W1", [IN_FEATS, H], F32, isOutput=False)
    W2_d = dp("W2", [H, H], F32, isOutput=False)
    A0_d = dp("A0", [H, H], F32, isOutput=False)
    A1_d = dp("A1", [H, H], F32, isOutput=False)
    A2_d = dp("A2", [H, H], F32, isOutput=False)
    Wm2_d = dp("Wm2", [H, NUM_CLASSES], F32, isOutput=False)
    b1_d = dp("b1", [H, 1], F32, isOutput=False)
    b2_d = dp("b2", [H, 1], F32, isOutput=False)
    bm1_d = dp("bm1", [H, 1], F32, isOutput=False)
    bm2_d = dp("bm2", [NUM_CLASSES, 1], F32, isOutput=False)
    out_d = dp("outT", [NUM_CLASSES, NPC_PAD], F32, isOutput=True)

    fs_in = [nc.dram_tensor(f"fs{r}_in", [NPC_PAD, H], TDT) for r in range(2)]
    fs_full = [nc.dram_tensor(f"fs{r}_full", [NTAB, H], TDT, addr_space="Shared")
               for r in range(2)]
    groups = [list(range(N_CORES))]

    with tile.TileContext(nc) as tc:
        with (
            tc.tile_pool(name="const", bufs=1) as cp,
            tc.tile_pool(name="big", bufs=1) as bp,
            tc.tile_pool(name="work", bufs=2) as wp,
            tc.tile_pool(name="ps", bufs=4, space="PSUM") as ps,
        ):
            # ---- constant loads ----
            W1_t = cp.tile([IN_FEATS, H], F32)
            nc.sync.dma_start(out=W1_t[:], in_=W1_d[:])
            W2_t = cp.tile([H, H], F32)
            nc.sync.dma_start(out=W2_t[:], in_=W2_d[:])
            A_t = []
            for i, d in enumerate((A0_d, A1_d, A2_d)):
                a = cp.tile([H, H], F32, name=f"A{i}_t")
                nc.sync.dma_start(out=a[:], in_=d[:])
                A_t.append(a)
            Wm2_t = cp.tile([H, NUM_CLASSES], F32)
            nc.sync.dma_start(out=Wm2_t[:], in_=Wm2_d[:])
            bias = {}
            for nm, d, pp in (("b1", b1_d, H), ("b2", b2_d, H),
                              ("bm1", bm1_d, H), ("bm2", bm2_d, NUM_CLASSES)):
                t = cp.tile([pp, 1], F32, name=f"{nm}_t")
                nc.sync.dma_start(out=t[:], in_=d[:])
                bias[nm] = t
            idx_t = cp.tile([P, tot_s], I32)
            nc.sync.dma_start(out=idx_t[:], in_=idx_d[:])
            ident = cp.tile([P, P], F32)
            make_identity(nc, ident[:])
            zero_t = cp.tile([P, H], TDT)
            nc.vector.memset(zero_t[:], 0.0)

            # dinv = 1/sqrt(max(deg,1))
            deg_t = cp.tile([P, S_ALL], F32)
            nc.sync.dma_start(out=deg_t[:], in_=deg_d[:])
            dinv = cp.tile([P, S_ALL], F32)
            nc.vector.tensor_scalar_max(deg_t[:], deg_t[:], 1.0)
            nc.scalar.sqrt(dinv[:], deg_t[:])
            nc.vector.reciprocal(dinv[:], dinv[:])

            # big buffers
            xT_t = bp.tile([P, NPC_PAD], F32)
            for j in range(4):
                w = NPC_PAD // 4
                nc.sync.dma_start(out=xT_t[:, j * w:(j + 1) * w],
                                  in_=xT_d[:, j * w:(j + 1) * w])
            h2 = bp.tile([P, S_ALL * H], F32)
            p1 = bp.tile([P, S_ALL * H], F32)
            p2 = bp.tile([P, S_ALL * H], F32)
            # slots covering stripe s are the prefix {k: s_k > s} (s_k is
            # non-increasing); K_of_s[s] is its length
            K_of_s = [int((s_k > s).sum()) for s in range(S_ALL)]

            for _rep in range(reps):

                # ---- phase 1: h2 = relu(relu(x@W1+b1)@W2+b2), fs0 = h2*dinv ----
                c0 = 0
                while c0 < NPC_PAD:
                    cw = min(512, NPC_PAD - c0)
                    ps1 = ps.tile([H, cw], F32, name="ps1", tag="mm")
                    nc.tensor.matmul(ps1[:], W1_t[:], xT_t[:, c0:c0 + cw],
                                     start=True, stop=True)
                    h1c = wp.tile([H, cw], F32, name="h1c")
                    nc.scalar.activation(h1c[:], ps1[:],
                                         mb.ActivationFunctionType.Relu,
                                         bias=bias["b1"][:, 0:1])
                    ps2 = ps.tile([H, cw], F32, name="ps2", tag="mm")
                    nc.tensor.matmul(ps2[:], W2_t[:], h1c[:], start=True, stop=True)
                    h2c = wp.tile([H, cw], F32, name="h2c")
                    nc.scalar.activation(h2c[:], ps2[:],
                                         mb.ActivationFunctionType.Relu,
                                         bias=bias["b2"][:, 0:1])
                    for s in range(cw // P):
                        gs = (c0 // P) + s
                        pst = ps.tile([P, H], F32, name="pst", tag="tr")
                        nc.tensor.transpose(pst[:], h2c[:, s * P:(s + 1) * P],
                                            ident[:H, :H])
                        nc.vector.tensor_copy(h2[:, gs * H:(gs + 1) * H], pst[:])
                        fst = wp.tile([P, H], TDT, name="fst")
                        nc.vector.tensor_scalar_mul(fst[:], pst[:],
                                                    dinv[:, gs:gs + 1])
                        nc.sync.dma_start(out=fs_in[0][gs * P:(gs + 1) * P, :],
                                          in_=fst[:])
                        if gs == S_ALL - 1 and NPC_PAD > NPC:
                            nc.sync.dma_start(
                                out=fs_in[0][NPC:NPC_PAD, :],
                                in_=zero_t[:NPC_PAD - NPC, :])
                    c0 += cw

                nc.gpsimd.collective_compute(
                    "AllGather", mb.AluOpType.bypass, replica_groups=groups,
                    ins=[fs_in[0][:]], outs=[fs_full[0][:]])

                # ---- rounds ----
                for rnd in range(2):
                    tab = fs_full[rnd]
                    p_prev = h2 if rnd == 0 else p1
                    p_out = p1 if rnd == 0 else p2
                    for gs0 in range(0, S_ALL, 2):
                        pair = [gs for gs in (gs0, gs0 + 1) if gs < S_ALL]
                        gbs = {}
                        for gs in pair:
                            gbs[gs] = bp.tile([P, K_of_s[0] * H], F32,
                                              name="gb", tag="gb", bufs=6)
                        kmaxp = max(K_of_s[gs] for gs in pair)
                        # interleave the two stripes' gathers so consecutive
                        # DMAs target different tiles
                        for k in range(kmaxp):
                            for gs in pair:
                                if k < K_of_s[gs]:
                                    nc.gpsimd.indirect_dma_start(
                                        out=gbs[gs][:, k * H:(k + 1) * H],
                                        out_offset=None,
                                        in_=tab[:],
                                        in_offset=bass.IndirectOffsetOnAxis(
                                            ap=idx_t[:, int(cum[k]) + gs:
                                                     int(cum[k]) + gs + 1],
                                            axis=0),
                                        compute_op=mb.AluOpType.bypass)
                        for gs in pair:
                            Kk = K_of_s[gs]
                            gb = gbs[gs]
                            sl = slice(gs * H, (gs + 1) * H)
                            dcol = dinv[:, gs:gs + 1]
                            red = wp.tile([P, H], F32, name="red")
                            nc.vector.tensor_reduce(
                                out=red[:],
                                in_=gb[:, :Kk * H].rearrange(
                                    "p (k f) -> p f k", f=H),
                                axis=mb.AxisListType.X, op=mb.AluOpType.add)
                            nc.vector.tensor_scalar_mul(red[:], red[:], dcol)
                            nc.vector.tensor_tensor(p_out[:, sl], p_prev[:, sl],
                                                    red[:],
                                                    op=mb.AluOpType.subtract)
                            if rnd == 0:
                                fst = wp.tile([P, H], TDT, name="fs1t")
                                nc.vector.tensor_scalar_mul(fst[:], p_out[:, sl],
                                                            dcol)
                                nc.sync.dma_start(
                                    out=fs_in[1][gs * P:(gs + 1) * P, :],
                                    in_=fst[:])
                                if gs == S_ALL - 1 and NPC_PAD > NPC:
                                    nc.sync.dma_start(
                                        out=fs_in[1][NPC:NPC_PAD, :],
                                        in_=zero_t[:NPC_PAD - NPC, :])
                    if rnd == 0:
                        nc.gpsimd.collective_compute(
                            "AllGather", mb.AluOpType.bypass, replica_groups=groups,
                            ins=[fs_in[1][:]], outs=[fs_full[1][:]])

                # ---- final: out = relu(h2@A0+p1@A1+p2@A2+bm1)@Wm2+bm2 ----
                for gs in range(S_ALL):
                    sl = slice(gs * H, (gs + 1) * H)
                    psy = ps.tile([H, P], F32, name="psy", tag="mm")
                    for i, srcbuf in enumerate((h2, p1, p2)):
                        pst = ps.tile([H, P], F32, name="ftr", tag="tr")
                        nc.tensor.transpose(pst[:], srcbuf[:, sl], ident[:])
                        rhs = wp.tile([H, P], F32, name="frhs")
                        nc.vector.tensor_copy(rhs[:], pst[:])
                        nc.tensor.matmul(psy[:], A_t[i][:], rhs[:],
                                         start=(i == 0), stop=(i == 2))
                    y2 = wp.tile([H, P], F32, name="y2")
                    nc.scalar.activation(y2[:], psy[:],
                                         mb.ActivationFunctionType.Relu,
                                         bias=bias["bm1"][:, 0:1])
                    pso = ps.tile([NUM_CLASSES, P], F32, name="pso", tag="mm")
                    nc.tensor.matmul(pso[:], Wm2_t[:], y2[:], start=True, stop=True)
                    ot = wp.tile([NUM_CLASSES, P], F32, name="ot")
                    nc.vector.tensor_scalar_add(ot[:], pso[:], bias["bm2"][:, 0:1])
                    nc.sync.dma_start(out=out_d[:, gs * P:(gs + 1) * P], in_=ot[:])

    return nc


# ---------------- execution (axon PJRT, 8 devices) ----------------
class _Exec:
    def __init__(self, nc):
        import jax
        from jax.sharding import Mesh, PartitionSpec, NamedSharding
        from jax.experimental.shard_map import shard_map
        _install_neff_cache()
        bass2jax.install_neuronx_cc_hook()
        self.jax = jax
        pn = nc.partition_id_tensor.name if nc.partition_id_tensor else None
        in_names, out_names, out_avals = [], [], []
        for alloc in nc.m.functions[0].allocations:
            if not isinstance(alloc, mb.MemoryLocationSet):
                continue
            name = alloc.memorylocations[0].name
            if alloc.kind == "ExternalInput":
                if name != pn:
                    in_names.append(name)
            elif alloc.kind == "ExternalOutput":
                out_names.append(name)
                out_avals.append(jax.core.ShapedArray(
                    tuple(alloc.tensor_shape), mb.dt.np(alloc.dtype)))
        self.in_names, self.out_names, self.out_avals = in_names, out_names, out_avals
        n_params, n_outs = len(in_names), len(out_avals)
        all_in = list(in_names) + list(out_names)
        if pn is not None:
            all_in.append(pn)

        def _body(*args):
            operands = list(args)
            if pn is not None:
                operands.append(bass2jax.partition_id_tensor())
            return tuple(bass2jax._bass_exec_p.bind(
                *operands, out_avals=tuple(out_avals), in_names=tuple(all_in),
                out_names=tuple(out_names), lowering_input_output_aliases=(),
                sim_require_finite=False, sim_require_nnan=False, nc=nc))

        devices = jax.devices()[:N_CORES]
        mesh = Mesh(np.asarray(devices), ("core",))
        self.fn = jax.jit(
            shard_map(_body, mesh=mesh,
                      in_specs=(PartitionSpec("core"),) * (n_params + n_outs),
                      out_specs=(PartitionSpec("core"),) * n_outs,
                      check_rep=False),
            donate_argnums=tuple(range(n_params, n_params + n_outs)),
            keep_unused=True)
        self.sharding = NamedSharding(mesh, PartitionSpec("core"))

    def put(self, in_maps):
        arrs = [np.concatenate([np.asarray(m[n]) for m in in_maps], axis=0)
                for n in self.in_names]
        return [self.jax.device_put(a, self.sharding) for a in arrs]

    def run(self, dev_in):
        zo = [self.jax.device_put(
            np.zeros((N_CORES * a.shape[0], *a.shape[1:]), a.dtype), self.sharding)
            for a in self.out_avals]
        outs = self.fn(*dev_in, *zo)
        self.jax.block_until_ready(outs)
        return outs

    def fetch(self, outs):
        return [np.asarray(o).reshape(N_CORES, *self.out_avals[i].shape)
                for i, o in enumerate(outs)]


_CACHE = {}


def _prepare(x, edge_index, W1, b1, W2, b2, Wm1, bm1, Wm2, bm2, reps=1,
             bf16=None):
    if bf16 is None:
        bf16 = bool(int(os.environ.get("BWGNN_BF16", "0")))
    pre = preprocess(x, edge_index)
    wts = host_weights(W1, b1, W2, b2, Wm1, bm1, Wm2, bm2)
    key = ("nc", pre["tot_s"], tuple(pre["s_k"].tolist()), reps, bf16)
    if key not in _CACHE:
        nc = build_nc(pre["s_k"], pre["cum"], pre["tot_s"], reps=reps, bf16=bf16)
        _split_waits(nc)
        _CACHE[key] = _Exec(nc)
    ex = _CACHE[key]
    in_maps = []
    for c in range(N_CORES):
        m = dict(xT=pre["xT"][c], degt=pre["degt"][c], idx=pre["idx"][c], **wts)
        in_maps.append(m)
    return ex, in_maps, pre


def kernel(x, edge_index, W1, b1, W2, b2, Wm1, bm1, Wm2, bm2):
    ex, in_maps, pre = _prepare(x, edge_index, W1, b1, W2, b2,
                                Wm1, bm1, Wm2, bm2)
    dev_in = ex.put(in_maps)
    outs = ex.run(dev_in)
    outT = ex.fetch(outs)[0]          # [N_CORES, 2, NPC_PAD]
    y = np.empty((N_NODES, NUM_CLASSES), dtype=np.float32)
    for c in range(N_CORES):
        y[c * NPC + pre["orders"][c]] = outT[c, :, :NPC].T
    return y


# revision 11
# speedup vs baseline: 31.7691x; 29.5586x over previous
"""BWGNN (Beta-Wavelet GNN) Trainium2 kernel — 8-core SPMD.

Math (exact refactoring of the reference):
  h  = relu(relu(x@W1+b1)@W2+b2)
  P(f) = f - dinv * segsum_dst((f*dinv)[src])          (dinv = clip(deg,1)^-0.5)
  All 3 Beta-Bernstein filters are polynomials of the same operator P applied
  to the same h, so only p1=P(h), p2=P(p1) are needed (2 message rounds, not 6):
    concat_i(sum_k theta_ik P^k h) @ Wm1 = h@A0 + p1@A1 + p2@A2,
    A_k = sum_i theta_ik * Wm1[64i:64(i+1)]            (host-precomputed)
  out = relu(h@A0 + p1@A1 + p2@A2 + bm1) @ Wm2 + bm2

Distribution: nodes block-sharded over 8 cores (12500/core); edges partitioned
by destination core; per-round AllGather of the scaled features (f*dinv) so
each core gathers source rows locally from its replicated table.

Message aggregation: per-core nodes are degree-sorted into "positions";
slot k holds the k-th edge of every node with deg>k (a position-prefix).
Each (slot, stripe-of-128-positions) is one canonical indirect DMA
(idx [128,1] -> rows [128,64]) with CCE fp32 accumulate into SBUF; padding
entries index a zeroed pad row of the table (adds exact zeros). Two buffer chains (even/odd slots) keep the
qPoolDynamic queue busy; a final add combines them.
"""
import hashlib
import math
import os
import time

import numpy as np

import concourse.bass as bass
import concourse.mybir as mb
import concourse.tile as tile
from concourse import bass2jax
from concourse.masks import make_identity

# ---------------- problem constants (hardcoded per contract) ----------------
N_NODES = 100000
N_EDGES = 1600000
IN_FEATS = 128
H = 64
NUM_CLASSES = 2
N_CORES = 8
NPC = N_NODES // N_CORES            # 12500 nodes per core
P = 128
S_ALL = (NPC + P - 1) // P          # 98 stripes
NPC_PAD = S_ALL * P                 # 12544
NTAB = N_CORES * NPC_PAD            # 100352 rows in the gathered table
SENTINEL = NPC                      # pad-row table index (those rows are zeroed)
F32 = mb.dt.float32
I32 = mb.dt.int32

THETAS = np.array([[3.0, -3.0, 0.75],
                   [0.0, 3.0, -1.5],
                   [0.0, 0.0, 0.75]])  # [filter, power]  (Beta-Bernstein, D=2)

_NEFF_CACHE_DIR = os.environ.get("BASS_NEFF_CACHE", "/tmp/neff_cache")


def _install_neff_cache():
    """Disk-cache walrus compiles by BIR hash (no cache in the stock hook)."""
    import concourse.bass_utils as bass_utils
    if getattr(bass2jax, "_neff_cache_installed", False):
        return
    orig = bass_utils.compile_bir_kernel

    def cached(bir_json, tmpdir, neff_name="file.neff"):
        os.makedirs(_NEFF_CACHE_DIR, exist_ok=True)
        key = hashlib.sha256(bir_json).hexdigest()[:32]
        path = os.path.join(_NEFF_CACHE_DIR, f"{key}.neff")
        if os.path.exists(path):
            dst = os.path.join(tmpdir, neff_name)
            with open(path, "rb") as f, open(dst, "wb") as g:
                g.write(f.read())
            return dst
        out = orig(bir_json, tmpdir, neff_name)
        try:
            with open(out, "rb") as f, open(path + ".tmp", "wb") as g:
                g.write(f.read())
            os.replace(path + ".tmp", path)
        except OSError:
            pass
        return out

    bass_utils.compile_bir_kernel = cached
    bass2jax.compile_bir_kernel = cached
    bass2jax._neff_cache_installed = True


# ---------------- walrus 1-wait-per-instruction workaround ----------------
def _split_waits(nc):
    """This walrus build rejects >1 sync wait per instruction; move excess
    waits onto no-fuse nops inserted just before, on the same engine."""
    for bb in nc.main_func.blocks:
        insts = list(bb.instructions)
        out, changed = [], False
        for inst in insts:
            si = inst.sync_info
            waits = list(si.on_wait) if si and si.on_wait else []
            if len(waits) > 1:
                for i, w in enumerate(waits[:-1]):
                    out.append(mb.InstNoOp(
                        name=f"{inst.name}-ws{i}", bass_nofuse=True,
                        engine=inst.engine,
                        sync_info=mb.SyncInfo(on_wait=[w], on_update=[])))
                si.on_wait = waits[-1:]
                inst.sync_info = si
                changed = True
            out.append(inst)
        if changed:
            bb.instructions = out


# ---------------- host-side preprocessing ----------------
def preprocess(x, edge_index):
    """Degree-sort nodes per core, build slot/stripe gather indices.

    Returns dict with per-core arrays + shared structure.
    """
    src = np.asarray(edge_index[0], dtype=np.int64)
    dst = np.asarray(edge_index[1], dtype=np.int64)
    x = np.asarray(x, dtype=np.float32)

    deg = np.bincount(dst, minlength=N_NODES).astype(np.int64)

    # per-core degree sort -> positions
    pos = np.empty(N_NODES, dtype=np.int64)
    orders = []
    for c in range(N_CORES):
        dc = deg[c * NPC:(c + 1) * NPC]
        order = np.argsort(-dc, kind="stable")      # position -> local node
        orders.append(order)
        pos[c * NPC + order] = np.arange(NPC)
    gid = (np.arange(N_NODES) // NPC) * NPC_PAD + pos   # node -> table row

    # edge rank within destination
    eorder = np.argsort(dst, kind="stable")
    ds = dst[eorder]
    first = np.r_[0, np.flatnonzero(np.diff(ds)) + 1]
    run_id = np.zeros(N_EDGES, dtype=np.int64)
    run_id[first[1:]] = 1
    run_id = np.cumsum(run_id)
    rank_sorted = np.arange(N_EDGES) - first[run_id]
    rank = np.empty(N_EDGES, dtype=np.int64)
    rank[eorder] = rank_sorted

    # shared slot structure (max over cores)
    deg_pc = deg.reshape(N_CORES, NPC)
    kmax = int(deg.max())
    ks = np.arange(kmax)
    n_k = (deg_pc[:, None, :] > ks[None, :, None]).sum(axis=2).max(axis=0)  # [kmax]
    s_k = np.maximum(1, (n_k + P - 1) // P).astype(np.int64)                # stripes per slot
    cum = np.r_[0, np.cumsum(s_k)]
    tot_s = int(cum[-1])

    # per-edge placement
    q = pos[dst]
    col = cum[rank] + q // P
    row = q % P
    core_e = dst // NPC
    idx_all = np.full((N_CORES, P, tot_s), SENTINEL, dtype=np.int32)
    idx_all[core_e, row, col] = gid[src].astype(np.int32)

    # per-core xT (position order, padded) and deg tile [128, S_ALL]
    xT = np.zeros((N_CORES, P, NPC_PAD), dtype=np.float32)
    degt = np.ones((N_CORES, P, S_ALL), dtype=np.float32)
    for c in range(N_CORES):
        xc = x[c * NPC:(c + 1) * NPC][orders[c]]          # [NPC, IN]
        xT[c, :, :NPC] = xc.T
        dp = np.ones(NPC_PAD, dtype=np.float32)
        dp[:NPC] = deg_pc[c][orders[c]]
        degt[c] = dp.reshape(S_ALL, P).T                  # deg at (p, s) = q=s*128+p

    return dict(idx=idx_all, xT=xT, degt=degt, s_k=s_k, cum=cum,
                tot_s=tot_s, kmax=kmax, orders=orders)


def host_weights(W1, b1, W2, b2, Wm1, bm1, Wm2, bm2):
    A = [sum(float(THETAS[i, k]) * np.asarray(Wm1, np.float32)[i * H:(i + 1) * H, :]
             for i in range(3)) for k in range(3)]
    return dict(
        W1=np.asarray(W1, np.float32), W2=np.asarray(W2, np.float32),
        A0=A[0].astype(np.float32), A1=A[1].astype(np.float32), A2=A[2].astype(np.float32),
        Wm2=np.asarray(Wm2, np.float32),
        b1=np.asarray(b1, np.float32).reshape(H, 1),
        b2=np.asarray(b2, np.float32).reshape(H, 1),
        bm1=np.asarray(bm1, np.float32).reshape(H, 1),
        bm2=np.asarray(bm2, np.float32).reshape(NUM_CLASSES, 1),
    )


# ---------------- device program ----------------
def build_nc(s_k, cum, tot_s, reps=1, bf16=False):
    """Build the SPMD Bass program. Slot structure (s_k stripes per slot) is
    compile-time static and identical on all cores.

    bf16=True stores the gathered feature tables (fs*) in bfloat16 — halves
    gather + allgather traffic; accumulation stays fp32 via CCE cast-add."""
    nc = bass.Bass()
    TDT = mb.dt.bfloat16 if bf16 else F32
    dp = nc.declare_dram_parameter
    xT_d = dp("xT", [P, NPC_PAD], F32, isOutput=False)
    deg_d = dp("degt", [P, S_ALL], F32, isOutput=False)
    idx_d = dp("idx", [P, tot_s], I32, isOutput=False)
    W1_d = dp("W1", [IN_FEATS, H], F32, isOutput=False)
    W2_d = dp("W2", [H, H], F32, isOutput=False)
    A0_d = dp("A0", [H, H], F32, isOutput=False)
    A1_d = dp("A1", [H, H], F32, isOutput=False)
    A2_d = dp("A2", [H, H], F32, isOutput=False)
    Wm2_d = dp("Wm2", [H, NUM_CLASSES], F32, isOutput=False)
    b1_d = dp("b1", [H, 1], F32, isOutput=False)
    b2_d = dp("b2", [H, 1], F32, isOutput=False)
    bm1_d = dp("bm1", [H, 1], F32, isOutput=False)
    bm2_d = dp("bm2", [NUM_CLASSES, 1], F32, isOutput=False)
    out_d = dp("outT", [NUM_CLASSES, NPC_PAD], F32, isOutput=True)

    fs_in = [nc.dram_tensor(f"fs{r}_in", [NPC_PAD, H], TDT) for r in range(2)]
    fs_full = [nc.dram_tensor(f"fs{r}_full", [NTAB, H], TDT, addr_space="Shared")
               for r in range(2)]
    groups = [list(range(N_CORES))]

    with tile.TileContext(nc) as tc:
        with (
            tc.tile_pool(name="const", bufs=1) as cp,
            tc.tile_pool(name="big", bufs=1) as bp,
            tc.tile_pool(name="work", bufs=2) as wp,
            tc.tile_pool(name="ps", bufs=4, space="PSUM") as ps,
        ):
            # ---- constant loads ----
            W1_t = cp.tile([IN_FEATS, H], F32)
            nc.sync.dma_start(out=W1_t[:], in_=W1_d[:])
            W2_t = cp.tile([H, H], F32)
            nc.sync.dma_start(out=W2_t[:], in_=W2_d[:])
            A_t = []
            for i, d in enumerate((A0_d, A1_d, A2_d)):
                a = cp.tile([H, H], F32, name=f"A{i}_t")
                nc.sync.dma_start(out=a[:], in_=d[:])
                A_t.append(a)
            Wm2_t = cp.tile([H, NUM_CLASSES], F32)
            nc.sync.dma_start(out=Wm2_t[:], in_=Wm2_d[:])
            bias = {}
            for nm, d, pp in (("b1", b1_d, H), ("b2", b2_d, H),
                              ("bm1", bm1_d, H), ("bm2", bm2_d, NUM_CLASSES)):
                t = cp.tile([pp, 1], F32, name=f"{nm}_t")
                nc.sync.dma_start(out=t[:], in_=d[:])
                bias[nm] = t
            idx_t = cp.tile([P, tot_s], I32)
            nc.sync.dma_start(out=idx_t[:], in_=idx_d[:])
            ident = cp.tile([P, P], F32)
            make_identity(nc, ident[:])
            zero_t = cp.tile([P, H], TDT)
            nc.vector.memset(zero_t[:], 0.0)

            # dinv = 1/sqrt(max(deg,1))
            deg_t = cp.tile([P, S_ALL], F32)
            nc.sync.dma_start(out=deg_t[:], in_=deg_d[:])
            dinv = cp.tile([P, S_ALL], F32)
            nc.vector.tensor_scalar_max(deg_t[:], deg_t[:], 1.0)
            nc.scalar.sqrt(dinv[:], deg_t[:])
            nc.vector.reciprocal(dinv[:], dinv[:])

            # big buffers
            xT_t = bp.tile([P, NPC_PAD], F32)
            for j in range(4):
                w = NPC_PAD // 4
                nc.sync.dma_start(out=xT_t[:, j * w:(j + 1) * w],
                                  in_=xT_d[:, j * w:(j + 1) * w])
            h2 = bp.tile([P, S_ALL * H], F32)
            p1 = bp.tile([P, S_ALL * H], F32)
            p2 = bp.tile([P, S_ALL * H], F32)
            # slots covering stripe s are the prefix {k: s_k > s} (s_k is
            # non-increasing); K_of_s[s] is its length
            K_of_s = [int((s_k > s).sum()) for s in range(S_ALL)]

            for _rep in range(reps):

                # ---- phase 1: h2 = relu(relu(x@W1+b1)@W2+b2), fs0 = h2*dinv ----
                c0 = 0
                while c0 < NPC_PAD:
                    cw = min(512, NPC_PAD - c0)
                    ps1 = ps.tile([H, cw], F32, name="ps1", tag="mm")
                    nc.tensor.matmul(ps1[:], W1_t[:], xT_t[:, c0:c0 + cw],
                                     start=True, stop=True)
                    h1c = wp.tile([H, cw], F32, name="h1c")
                    nc.scalar.activation(h1c[:], ps1[:],
                                         mb.ActivationFunctionType.Relu,
                                         bias=bias["b1"][:, 0:1])
                    ps2 = ps.tile([H, cw], F32, name="ps2", tag="mm")
                    nc.tensor.matmul(ps2[:], W2_t[:], h1c[:], start=True, stop=True)
                    h2c = wp.tile([H, cw], F32, name="h2c")
                    nc.scalar.activation(h2c[:], ps2[:],
                                         mb.ActivationFunctionType.Relu,
                                         bias=bias["b2"][:, 0:1])
                    for s in range(cw // P):
                        gs = (c0 // P) + s
                        pst = ps.tile([P, H], F32, name="pst", tag="tr")
                        nc.tensor.transpose(pst[:], h2c[:, s * P:(s + 1) * P],
                                            ident[:H, :H])
                        nc.vector.tensor_copy(h2[:, gs * H:(gs + 1) * H], pst[:])
                        fst = wp.tile([P, H], TDT, name="fst")
                        nc.vector.tensor_scalar_mul(fst[:], pst[:],
                                                    dinv[:, gs:gs + 1])
                        nc.sync.dma_start(out=fs_in[0][gs * P:(gs + 1) * P, :],
                                          in_=fst[:])
                        if gs == S_ALL - 1 and NPC_PAD > NPC:
                            nc.sync.dma_start(
                                out=fs_in[0][NPC:NPC_PAD, :],
                                in_=zero_t[:NPC_PAD - NPC, :])
                    c0 += cw

                nc.gpsimd.collective_compute(
                    "AllGather", mb.AluOpType.bypass, replica_groups=groups,
                    ins=[fs_in[0][:]], outs=[fs_full[0][:]])

                # ---- rounds ----
                for rnd in range(2):
                    tab = fs_full[rnd]
                    p_prev = h2 if rnd == 0 else p1
                    p_out = p1 if rnd == 0 else p2
                    for gs0 in range(0, S_ALL, 2):
                        pair = [gs for gs in (gs0, gs0 + 1) if gs < S_ALL]
                        gbs = {}
                        for gs in pair:
                            gbs[gs] = bp.tile([P, K_of_s[0] * H], F32,
                                              name="gb", tag="gb", bufs=6)
                        kmaxp = max(K_of_s[gs] for gs in pair)
                        # interleave the two stripes' gathers so consecutive
                        # DMAs target different tiles
                        for k in range(kmaxp):
                            for gs in pair:
                                if k < K_of_s[gs]:
                                    nc.gpsimd.indirect_dma_start(
                                        out=gbs[gs][:, k * H:(k + 1) * H],
                                        out_offset=None,
                                        in_=tab[:],
                                        in_offset=bass.IndirectOffsetOnAxis(
                                            ap=idx_t[:, int(cum[k]) + gs:
                                                     int(cum[k]) + gs + 1],
                                            axis=0),
                                        compute_op=mb.AluOpType.bypass)
                        for gs in pair:
                            Kk = K_of_s[gs]
                            gb = gbs[gs]
                            sl = slice(gs * H, (gs + 1) * H)
                            dcol = dinv[:, gs:gs + 1]
                            red = wp.tile([P, H], F32, name="red")
                            nc.vector.tensor_reduce(
                                out=red[:],
                                in_=gb[:, :Kk * H].rearrange(
                                    "p (k f) -> p f k", f=H),
                                axis=mb.AxisListType.X, op=mb.AluOpType.add)
                            nc.vector.tensor_scalar_mul(red[:], red[:], dcol)
                            nc.vector.tensor_tensor(p_out[:, sl], p_prev[:, sl],
                                                    red[:],
                                                    op=mb.AluOpType.subtract)
                            if rnd == 0:
                                fst = wp.tile([P, H], TDT, name="fs1t")
                                nc.vector.tensor_scalar_mul(fst[:], p_out[:, sl],
                                                            dcol)
                                nc.sync.dma_start(
                                    out=fs_in[1][gs * P:(gs + 1) * P, :],
                                    in_=fst[:])
                                if gs == S_ALL - 1 and NPC_PAD > NPC:
                                    nc.sync.dma_start(
                                        out=fs_in[1][NPC:NPC_PAD, :],
                                        in_=zero_t[:NPC_PAD - NPC, :])
                    if rnd == 0:
                        nc.gpsimd.collective_compute(
                            "AllGather", mb.AluOpType.bypass, replica_groups=groups,
                            ins=[fs_in[1][:]], outs=[fs_full[1][:]])

                # ---- final: out = relu(h2@A0+p1@A1+p2@A2+bm1)@Wm2+bm2 ----
                for gs in range(S_ALL):
                    sl = slice(gs * H, (gs + 1) * H)
                    psy = ps.tile([H, P], F32, name="psy", tag="mm")
                    for i, srcbuf in enumerate((h2, p1, p2)):
                        pst = ps.tile([H, P], F32, name="ftr", tag="tr")
                        nc.tensor.transpose(pst[:], srcbuf[:, sl], ident[:])
                        rhs = wp.tile([H, P], F32, name="frhs")
                        nc.vector.tensor_copy(rhs[:], pst[:])
                        nc.tensor.matmul(psy[:], A_t[i][:], rhs[:],
                                         start=(i == 0), stop=(i == 2))
                    y2 = wp.tile([H, P], F32, name="y2")
                    nc.scalar.activation(y2[:], psy[:],
                                         mb.ActivationFunctionType.Relu,
                                         bias=bias["bm1"][:, 0:1])
                    pso = ps.tile([NUM_CLASSES, P], F32, name="pso", tag="mm")
                    nc.tensor.matmul(pso[:], Wm2_t[:], y2[:], start=True, stop=True)
                    ot = wp.tile([NUM_CLASSES, P], F32, name="ot")
                    nc.vector.tensor_scalar_add(ot[:], pso[:], bias["bm2"][:, 0:1])
                    nc.sync.dma_start(out=out_d[:, gs * P:(gs + 1) * P], in_=ot[:])

    return nc


# ---------------- execution (axon PJRT, 8 devices) ----------------
class _Exec:
    def __init__(self, nc):
        import jax
        from jax.sharding import Mesh, PartitionSpec, NamedSharding
        from jax.experimental.shard_map import shard_map
        _install_neff_cache()
        bass2jax.install_neuronx_cc_hook()
        self.jax = jax
        pn = nc.partition_id_tensor.name if nc.partition_id_tensor else None
        in_names, out_names, out_avals = [], [], []
        for alloc in nc.m.functions[0].allocations:
            if not isinstance(alloc, mb.MemoryLocationSet):
                continue
            name = alloc.memorylocations[0].name
            if alloc.kind == "ExternalInput":
                if name != pn:
                    in_names.append(name)
            elif alloc.kind == "ExternalOutput":
                out_names.append(name)
                out_avals.append(jax.core.ShapedArray(
                    tuple(alloc.tensor_shape), mb.dt.np(alloc.dtype)))
        self.in_names, self.out_names, self.out_avals = in_names, out_names, out_avals
        n_params, n_outs = len(in_names), len(out_avals)
        all_in = list(in_names) + list(out_names)
        if pn is not None:
            all_in.append(pn)

        def _body(*args):
            operands = list(args)
            if pn is not None:
                operands.append(bass2jax.partition_id_tensor())
            return tuple(bass2jax._bass_exec_p.bind(
                *operands, out_avals=tuple(out_avals), in_names=tuple(all_in),
                out_names=tuple(out_names), lowering_input_output_aliases=(),
                sim_require_finite=False, sim_require_nnan=False, nc=nc))

        devices = jax.devices()[:N_CORES]
        mesh = Mesh(np.asarray(devices), ("core",))
        self.fn = jax.jit(
            shard_map(_body, mesh=mesh,
                      in_specs=(PartitionSpec("core"),) * (n_params + n_outs),
                      out_specs=(PartitionSpec("core"),) * n_outs,
                      check_rep=False),
            donate_argnums=tuple(range(n_params, n_params + n_outs)),
            keep_unused=True)
        self.sharding = NamedSharding(mesh, PartitionSpec("core"))

    def put(self, in_maps):
        arrs = [np.concatenate([np.asarray(m[n]) for m in in_maps], axis=0)
                for n in self.in_names]
        return [self.jax.device_put(a, self.sharding) for a in arrs]

    def run(self, dev_in):
        zo = [self.jax.device_put(
            np.zeros((N_CORES * a.shape[0], *a.shape[1:]), a.dtype), self.sharding)
            for a in self.out_avals]
        outs = self.fn(*dev_in, *zo)
        self.jax.block_until_ready(outs)
        return outs

    def fetch(self, outs):
        return [np.asarray(o).reshape(N_CORES, *self.out_avals[i].shape)
                for i, o in enumerate(outs)]


_CACHE = {}


def _prepare(x, edge_index, W1, b1, W2, b2, Wm1, bm1, Wm2, bm2, reps=1,
             bf16=None):
    if bf16 is None:
        bf16 = bool(int(os.environ.get("BWGNN_BF16", "1")))
    pre = preprocess(x, edge_index)
    wts = host_weights(W1, b1, W2, b2, Wm1, bm1, Wm2, bm2)
    key = ("nc", pre["tot_s"], tuple(pre["s_k"].tolist()), reps, bf16)
    if key not in _CACHE:
        nc = build_nc(pre["s_k"], pre["cum"], pre["tot_s"], reps=reps, bf16=bf16)
        _split_waits(nc)
        _CACHE[key] = _Exec(nc)
    ex = _CACHE[key]
    in_maps = []
    for c in range(N_CORES):
        m = dict(xT=pre["xT"][c], degt=pre["degt"][c], idx=pre["idx"][c], **wts)
        in_maps.append(m)
    return ex, in_maps, pre


def kernel(x, edge_index, W1, b1, W2, b2, Wm1, bm1, Wm2, bm2):
    ex, in_maps, pre = _prepare(x, edge_index, W1, b1, W2, b2,
                                Wm1, bm1, Wm2, bm2)
    dev_in = ex.put(in_maps)
    outs = ex.run(dev_in)
    outT = ex.fetch(outs)[0]          # [N_CORES, 2, NPC_PAD]
    y = np.empty((N_NODES, NUM_CLASSES), dtype=np.float32)
    for c in range(N_CORES):
        y[c * NPC + pre["orders"][c]] = outT[c, :, :NPC].T
    return y
